# revision 2
# baseline (speedup 1.0000x reference)
"""Trainium2 Bass kernel for nn_MHParallelAttention (B=4,S=1024,H=16,DK=64).

Sharding: 8 cores = (batch) x (query-row half); each core owns output rows
[b, s0:s0+512, :] end-to-end, no collectives.

all-bf16 datapath (validated end-to-end rel-err ~5.5e-3 vs 2e-2 gate):
  * inputs qk/mask/weights DMA'd as bf16 (halves HBM traffic vs f32r),
    f32 PSUM accumulation everywhere; biases/Wc kept f32 in a tiny aux
    tensor (DVE scalar operands are exempt from the 2x-mode dtype rule).
  * per-j merged [kT|qT] input DMA (one 3KB/partition descriptor chain).
  * k-proj PSUM is one [128,1024] 2-bank tile -> single 1024-elem tanh.
  * DVE ops (Wc scale, mask-mult+accum, normalize) all bf16 SBUF -> 4x mode.
  * output written bf16, upconverted host-side.

Algebra folds (as baseline): scores+head-combine collapse to one
[512,1024]@[1024,1024]^T matmul accumulated over 8 feature chunks; bc
dropped (softmax shift-invariant); block-diagonal head-pair projections;
softmax without max-subtraction; 0/1 mask multiplied after exp, fused
with the row-sum in one DVE op.
"""

import os
import sys

import numpy as np

for _p in ("/opt/trn_rl_repo", "/root/.axon_site/_ro/trn_rl_repo"):
    if os.path.isdir(_p) and _p not in sys.path:
        sys.path.insert(0, _p)

import ml_dtypes

import concourse.bass as bass
import concourse.mybir as mybir
import concourse.tile as tile
from concourse import bacc
from concourse.bass import ds, ts

H, DK = 16, 64
B, S = 4, 1024
SQ = 512
NCORES = 8
NJ = 8

F32 = mybir.dt.float32
BF16 = mybir.dt.bfloat16
BF16NP = ml_dtypes.bfloat16


def build_nc():
    nc = bacc.Bacc(None, target_bir_lowering=False, debug=False)

    qk = nc.dram_tensor("qk", [NJ, 128, 1536], BF16, kind="ExternalInput")
    msk = nc.dram_tensor("msk", [2, 128, 2048], BF16, kind="ExternalInput")
    wts = nc.dram_tensor("wts", [128, 256], BF16, kind="ExternalInput")
    aux = nc.dram_tensor("aux", [128, 10], F32, kind="ExternalInput")
    out = nc.dram_tensor("out", [SQ, S], BF16, kind="ExternalOutput")

    Tanh = mybir.ActivationFunctionType.Tanh
    Exp = mybir.ActivationFunctionType.Exp

    with tile.TileContext(nc) as tc:
        with (
            tc.tile_pool(name="const", bufs=1) as cst,
            tc.tile_pool(name="qkin", bufs=3) as qkin,
            tc.tile_pool(name="kpp", bufs=1) as kpp,
            tc.tile_pool(name="qpp", bufs=1) as qpp,
            tc.tile_pool(name="tmp", bufs=2) as tmpp,
            tc.tile_pool(name="mrow", bufs=1) as mrp,
            tc.tile_pool(name="soft", bufs=4) as softp,
            tc.tile_pool(name="stat", bufs=8) as statp,
            tc.tile_pool(name="obuf", bufs=4) as obp,
            tc.tile_pool(name="pproj", bufs=1, space="PSUM") as pproj,
            tc.tile_pool(name="pscore", bufs=5, space="PSUM") as pscore,
        ):
            # weights + aux ride the idle Pool/SWDGE queue FIRST: no HWDGE
            # slot, and their transfers land before the first qk chunk
            # clears the SP path — the weights gate the first matmul
            wts_sb = cst.tile([128, 256], BF16, tag="wts")
            aux_sb = cst.tile([128, 10], F32, tag="aux")
            nc.gpsimd.dma_start(out=wts_sb[:], in_=wts[:])
            nc.gpsimd.dma_start(out=aux_sb[:], in_=aux[:])

            # dummy activation on a memset tile: hoists the 1283ns
            # activation-table load to t~0, off the first-tanh critical path
            warm = cst.tile([128, 1], F32, tag="warm")
            nc.gpsimd.memset(warm[:], 0.0)
            nc.scalar.activation(warm[:], warm[:],
                                 mybir.ActivationFunctionType.Tanh)

            # PE p-state warmup: a burst of tiny matmuls starting at t~0
            # ramps the tensor engine to full clock before the first real
            # projection arrives (the ramp state survives the idle gap)
            wmm = cst.tile([128, 128], BF16, tag="wmm")
            nc.gpsimd.memset(wmm[:], 0.0)
            wps = pproj.tile([128, 1024], F32, tag="pk", name="warmps")
            for _ in range(14):
                nc.tensor.matmul(wps[:, 0:64], wmm[:], wmm[:, 0:64])
            wrd = cst.tile([128, 1], F32, tag="wrd")
            nc.vector.tensor_copy(out=wrd[:], in_=wps[:, 0:1])
            wkb = wts_sb[:, 0:128]
            wqb = wts_sb[:, 128:256]
            bkb = aux_sb[:, 0:1]
            bqb = aux_sb[:, 1:2]
            wcb = aux_sb[:, 2:10]

            mk = mrp.tile([128, 2, 2048], BF16, tag="mk")

            kp = [kpp.tile([128, S], BF16, tag=f"kp{j}", name=f"kp{j}")
                  for j in range(NJ)]
            qp = [qpp.tile([128, SQ], BF16, tag=f"qp{j}", name=f"qp{j}")
                  for j in range(NJ)]

            # 5 inline score chunks (4x [t01,kh] + t2kh0) fill the 5 spare
            # PSUM banks; the 3 remaining chunks run in phase 2.
            INLINE = [(0, 0), (0, 1), (1, 0), (1, 1), (2, 0)]
            pst = {c: pscore.tile([128, 512], F32, tag="ps", bufs=5,
                   name=f"psA_{c[0]}_{c[1]}") for c in INLINE}

            # ---- phase 1: per-j input DMA -> projections -> inline scores
            for j in range(NJ):
                qkt = qkin.tile([128, 1536], BF16, tag="qk", name=f"qk{j}")
                if j == 0:
                    # k-chunk first, q-chunk split off so the k-projection
                    # starts as early as possible (weights come via SWDGE)
                    nc.sync.dma_start(out=qkt[:, 0:1024], in_=qk[0, :, 0:1024])
                    nc.sync.dma_start(out=qkt[:, 1024:1536],
                                      in_=qk[0, :, 1024:1536])
                else:
                    nc.sync.dma_start(out=qkt[:], in_=qk[j])
                pk = pproj.tile([128, 1024], F32, tag="pk")
                nc.tensor.matmul(pk[:, 0:512], wkb, qkt[:, 0:512])
                nc.tensor.matmul(pk[:, 512:1024], wkb, qkt[:, 512:1024])
                pq = pproj.tile([128, 512], F32, tag="pq")
                nc.tensor.matmul(pq[:], wqb, qkt[:, 1024:1536])
                nc.scalar.activation(kp[j][:], pk[:], Tanh, bias=bkb)
                tq = tmpp.tile([128, SQ], BF16, tag="tq")
                nc.scalar.activation(tq[:], pq[:], Tanh, bias=bqb)
                nc.vector.tensor_scalar_mul(qp[j][:], tq[:], wcb[:, j:j + 1])
                for (t, kh) in INLINE:
                    nc.tensor.matmul(
                        pst[(t, kh)][:], qp[j][:, ts(t, 128)],
                        kp[j][:, ts(kh, 512)],
                        start=(j == 0), stop=(j == NJ - 1),
                    )

            # mask halves after inputs on the same queue (tail-only consumers)
            nc.sync.dma_start(out=mk[:, 0], in_=msk[0])
            nc.sync.dma_start(out=mk[:, 1], in_=msk[1])

            def tail_chain(t, psa, psb, oq=None, merged_out=False):
                ex = softp.tile([128, S], BF16, tag="ex")
                exm = obp.tile([128, S], BF16, tag="exm")
                s0 = statp.tile([128, 1], F32, tag="s0")
                s1 = statp.tile([128, 1], F32, tag="s1")
                mrow = mk[:, t // 2, ds((t % 2) * 1024, 1024)]
                nc.scalar.activation(ex[:, ts(0, 512)], psa[:], Exp)
                nc.vector.scalar_tensor_tensor(
                    exm[:, ts(0, 512)], ex[:, ts(0, 512)], 1.0,
                    mrow[:, ts(0, 512)],
                    op0=mybir.AluOpType.bypass, op1=mybir.AluOpType.mult,
                    accum_out=s0[:],
                )
                nc.scalar.activation(ex[:, ts(1, 512)], psb[:], Exp)
                nc.vector.scalar_tensor_tensor(
                    exm[:, ts(1, 512)], ex[:, ts(1, 512)], 1.0,
                    mrow[:, ts(1, 512)],
                    op0=mybir.AluOpType.bypass, op1=mybir.AluOpType.mult,
                    accum_out=s1[:],
                )
                ssum = statp.tile([128, 1], F32, tag="ssum")
                nc.vector.tensor_tensor(ssum[:], s0[:], s1[:],
                                        op=mybir.AluOpType.add)
                rec = statp.tile([128, 1], F32, tag="rec")
                nc.vector.reciprocal(rec[:], ssum[:])
                ot = obp.tile([128, S], BF16, tag="ot")
                oq = oq or nc.sync
                if merged_out:
                    nc.vector.tensor_scalar_mul(ot[:], exm[:], rec[:])
                    oq.dma_start(out=out[ts(t, 128), :], in_=ot[:])
                else:
                    for hh in range(2):
                        nc.vector.tensor_scalar_mul(
                            ot[:, ts(hh, 512)], exm[:, ts(hh, 512)], rec[:])
                        oq.dma_start(
                            out=out[ts(t, 128), ds(hh * 512, 512)],
                            in_=ot[:, ts(hh, 512)])

            # t=0,1 finished in phase 1 -> chain + output immediately
            for t in range(2):
                tail_chain(t, pst[(t, 0)], pst[(t, 1)], merged_out=True)

            # ---- phase 2: remaining chunks t2kh1, t3kh0, t3kh1 go into the
            # DEAD projection PSUM banks (free right after j=7's tanhs), so
            # no score matmul waits on a softmax-tail read to recycle a bank.
            # Each chunk fully accumulates before the next starts so its
            # tail overlaps the following chunk's matmuls.
            def score_chunk(t, kh, ps):
                for j in range(NJ):
                    nc.tensor.matmul(
                        ps[:], qp[j][:, ts(t, 128)], kp[j][:, ts(kh, 512)],
                        start=(j == 0), stop=(j == NJ - 1),
                    )
                return ps

            psx = pproj.tile([128, 1024], F32, tag="pk", name="psB_pk")
            psq = pproj.tile([128, 512], F32, tag="pq", name="psB_pq")
            ps21 = score_chunk(2, 1, psx[:, 0:512])
            # t2's outputs ride the Pool/SWDGE queue: no HWDGE slot needed,
            # so t3's output DMAs aren't stuck behind them
            tail_chain(2, pst[(2, 0)], ps21, oq=nc.gpsimd, merged_out=True)
            # t3: one 512 chunk + two 256 chunks; the 256s shorten the last
            # exp/mask-mult on the final serial path. Matmul groups on one
            # PSUM tile serialize, so the 256s (sharing psx with ps21) run
            # last while the 512 chunk gets the independent psq tile.
            regions = [(psx[:, 512:1024], ds(0, 512)),
                       (psq[:, 0:256], ds(512, 256)),
                       (psq[:, 256:512], ds(768, 256))]
            for ps, cs in regions:
                for j in range(NJ):
                    nc.tensor.matmul(
                        ps, qp[j][:, ts(3, 128)], kp[j][:, cs],
                        start=(j == 0), stop=(j == NJ - 1),
                    )

            # ---- t3 tail, fine-grained: outputs on the ACT queue (its SEQ
            # is idle at the tail, no serialization behind t2's outputs)
            ex = softp.tile([128, S], BF16, tag="ex")
            exm = obp.tile([128, S], BF16, tag="exm")
            mrow = mk[:, 1, ds(1024, 1024)]
            sc = [statp.tile([128, 1], F32, tag=f"sc{ci}", name=f"sc{ci}")
                  for ci in range(3)]
            for ci, (ps, cs) in enumerate(regions):
                nc.scalar.activation(ex[:, cs], ps, Exp)
                nc.vector.scalar_tensor_tensor(
                    exm[:, cs], ex[:, cs], 1.0, mrow[:, cs],
                    op0=mybir.AluOpType.bypass, op1=mybir.AluOpType.mult,
                    accum_out=sc[ci][:],
                )
            s01 = statp.tile([128, 1], F32, tag="s01")
            nc.vector.tensor_tensor(s01[:], sc[0][:], sc[1][:],
                                    op=mybir.AluOpType.add)
            ssum = statp.tile([128, 1], F32, tag="ssum")
            nc.vector.tensor_tensor(ssum[:], s01[:], sc[2][:],
                                    op=mybir.AluOpType.add)
            rec = statp.tile([128, 1], F32, tag="rec")
            nc.vector.reciprocal(rec[:], ssum[:])
            ot = obp.tile([128, S], BF16, tag="ot")
            nc.vector.tensor_scalar_mul(ot[:], exm[:], rec[:])
            nc.scalar.dma_start(out=out[ts(3, 128), :], in_=ot[:])

    nc.compile()
    return nc


_NC = None


def _get_nc():
    global _NC
    if _NC is None:
        _NC = build_nc()
    return _NC


def make_in_maps(query, key, mask, Wq, bq, Wk, bk, Wc, bc):
    query = np.asarray(query, np.float32)
    key = np.asarray(key, np.float32)
    mask = np.asarray(mask)
    Wq = np.asarray(Wq, np.float32)
    Wk = np.asarray(Wk, np.float32)
    Wc = np.asarray(Wc, np.float32)
    bq = np.asarray(bq, np.float32)
    bk = np.asarray(bk, np.float32)

    def blockdiag(W):
        blk = np.zeros((128, 128), np.float32)
        blk[0:64, 0:64] = W.T
        blk[64:128, 64:128] = W.T
        return blk

    wts = np.zeros((128, 256), np.float32)
    wts[:, 0:128] = blockdiag(Wk)
    wts[:, 128:256] = blockdiag(Wq)
    wts = wts.astype(BF16NP)

    aux = np.zeros((128, 10), np.float32)
    aux[:, 0] = np.tile(bk.reshape(-1), 2)
    aux[:, 1] = np.tile(bq.reshape(-1), 2)
    for j in range(NJ):
        aux[0:64, 2 + j] = Wc[0, 2 * j]
        aux[64:128, 2 + j] = Wc[0, 2 * j + 1]

    in_maps = []
    for c in range(NCORES):
        b, half = divmod(c, 2)
        s0 = half * SQ
        kh_ = key[b].reshape(H, S, DK)
        kTc = np.ascontiguousarray(kh_.transpose(0, 2, 1)).reshape(NJ, 128, S)
        qh = query[b].reshape(H, S, DK)[:, s0:s0 + SQ, :]
        qTc = np.ascontiguousarray(qh.transpose(0, 2, 1)).reshape(NJ, 128, SQ)
        qkc = np.concatenate([kTc, qTc], axis=2).astype(BF16NP)
        mv = (mask[b, s0:s0 + SQ, :] != 0).astype(BF16NP)
        mc = np.ascontiguousarray(
            mv.reshape(2, 2, 128, S).transpose(0, 2, 1, 3).reshape(2, 128, 2048))
        in_maps.append({"qk": qkc, "msk": mc, "wts": wts, "aux": aux})
    return in_maps


def kernel(query, key, mask, Wq, bq, Wk, bk, Wc, bc):
    from concourse.bass_utils import run_bass_kernel_spmd

    nc = _get_nc()
    in_maps = make_in_maps(query, key, mask, Wq, bq, Wk, bk, Wc, bc)
    res = run_bass_kernel_spmd(nc, in_maps, list(range(NCORES)))
    full = np.empty((B, S, S), np.float32)
    for c in range(NCORES):
        b, half = divmod(c, 2)
        full[b, half * SQ:(half + 1) * SQ, :] = \
            np.asarray(res.results[c]["out"]).astype(np.float32)
    return full
